# revision 25
# baseline (speedup 1.0000x reference)
"""KLDiscretLoss joints kernel for TRN2 (8 NeuronCores, Bass/Tile).

Math: for each row (b,j,d) of BINS logits,
  kl_row_sum = sum_bins labels*(log_labels - log_scores)
             = w/St + log(So) - log(St)
  where St = sum(exp(t)), So = sum(exp(o)), w = sum(exp(t)*(t-o)).
(no max-subtraction needed: randn inputs, |x| <~ 6, exp is safe in f32)

Sharding: data-parallel over batch, 32 batches/core -> 1088 rows/core.
Device streams both tensors once (memory-bound, ~49.5us DMA roofline
per core) and emits per-row partial stats; host does the per-row
combine + batch-mean + sum-over-d + min-over-j in float64.

Schedule notes (TimelineSim cost model: 55412ns vs 56782ns baseline;
breakdown: 1966 prologue + 49514 gapless stream + 2231 compute drain
+ 107 trigger/store + 900 DMA-sem prop + 694 exit barrier):
- w's multiply+row-sum is ONE custom-DVE op (dve_ops.TENSOR_TENSOR_
  REDUCE uop: out = in0*in1, accum_out = row sum). The native fused
  TENSOR_TENSOR_REDUCE ISA opcode crashes this HW path
  (NRT_EXEC_UNIT_UNRECOVERABLE) but the CUSTOM_DVE_ANT uop-engine
  version runs fine and halves DVE cost vs separate mul+reduce.
- Bulk chunks ("pool"): GpSimd computes t-o, DVE does the fused
  mul+reduce. Tail chunks ("wsplit"): w = sum(et*t) - sum(et*o) as
  two fused ops -- the t-side runs as soon as t lands (+900ns DMA
  sem), so after the final o only sum(et*o) + exp(o) remain.
- The stats store is a SWDGE prepare+trigger kv_writeback (a plain
  SBUF->DRAM [128, ncols] copy; batch=1/dhi=128/dho=1/ncn=n_ctx).
  Descriptors encode ADDRESSES only, so desc-gen (~1us fixed SWDGE
  cost) is legal before the data exists; Tile would still gate the
  prep on big's writers, so the prep instead reads store_src, a
  manual-range ALIAS of big's bytes that nothing ever writes
  (bump-allocated and manual tensors are not range-fenced against
  each other). The prep then has no data deps and desc-gen runs ~1us
  into the program. After the last compute only ~107ns remain
  (gating EventSemaphore + Pool trigger decode + 9ns transfer) vs
  ~1330ns for a HWDGE dma_start store or ~1100ns for a data-gated
  prep.
- The trigger is ordered after the final ACT exp and final DVE cTTR
  via signals_writable on the last eo/prod work tiles (WAW edges ->
  EventSemaphore waits on both engines' final ticks; in-order
  engines make those cover ALL stats writers). Never pass a tensor
  the prep reads to signals_writable (self-deadlock: the trigger
  would wait on the prep's DMASW read-tick, which only fires when
  the trigger itself fires). sem= must be Tile's DMASW0 lane
  semaphore (tc.sems[...]) or the exit barrier deadlocks. accum_out
  OVERWRITES, so big needs no memset (runt rows 64:128 of its
  columns are garbage; never read by the host decode).
- Tail taper [800, 480, 768] + t12-before-o11 stream order are from
  a local search; ACT (exp+accum: 0.833ns/col + 372ns/op) and DVE
  (cTTR: ~1.04ns/col + ~105ns/op) are both ~saturated over the last
  ~6us of the stream, so the ~2.2us drain is the joint throughput
  floor of the two engines, not a scheduling gap.
- The runt tile (64 rows) is processed early where engines have
  slack: engine cost scales with free size (columns), not rows, so a
  late runt is maximum compute per streamed byte -- the worst tail.
- bufs io=6/work=3 keep the 49.5us DMA stream completely gapless.
"""

import numpy as np

import concourse.bass as bass
import concourse.tile as tile
from concourse import bacc, mybir
from concourse.bass_utils import run_bass_kernel_spmd
from concourse.dve_ops import TENSOR_TENSOR_REDUCE
from concourse.tile_sem_assignment import PROC_NAME_TO_IDX

B, J, D, BINS = 256, 17, 2, 2048
NCORES = 8
BS = B // NCORES
ROWS = BS * J * D              # 1088 rows per core
P = 128
F32 = mybir.dt.float32
I32 = mybir.dt.int32
Exp = mybir.ActivationFunctionType.Exp

# Chunk spec: (r0, R, c0, CS, mode[, flags]). Process order = list order.
#   "pool":   sub on GpSimd, fused mul+reduce on DVE   (cols: St,So,w)
#   "wsplit": w = ttr(et,t) - ttr(et,o), no sub        (cols: St,So,wt,wo)
# flags: pre_t  = emit exp(t) (+wt unless pre_wt=False) at t-DMA issue
#        pre_wt = False defers the wt cTTR to the chunk's main emission
_PRE = dict(pre_t=True)
CHUNKS = [
    (0,    128, 0, 2048, "pool"),
    (1024,  64, 0, 2048, "pool"),      # runt early: engines have slack
    (128,  128, 0, 2048, "pool"),
    (256,  128, 0, 2048, "pool"),
    (384,  128, 0, 2048, "pool"),
    (512,  128, 0, 2048, "pool"),
    (640,  128, 0,    1024, "pool"),
    (640,  128, 1024, 1024, "pool"),
    (768,  128, 0,    1024, "wsplit", _PRE),
    (768,  128, 1024, 1024, "wsplit", _PRE),
    (896,  128, 0,    800,  "wsplit", _PRE),
    (896,  128, 800,  480,  "wsplit", _PRE),
    (896,  128, 1280, 768,  "wsplit", dict(pre_t=True, pre_wt=False)),
]
BUFS_IO = 6
BUFS_WORK = 3
# Hoisting tail-chunk t-loads to the stream front was measured WORSE
# (+230..+470ns): the stream is a fixed-rate pipe, so early hoisted loads
# displace chunks 8-10's arrivals INTO the saturated tail window, costing
# more than the removed wt cTTRs save. Keep empty.
HOIST = ()

_cache = {}


def _flags(chunk):
    return chunk[5] if len(chunk) > 5 else {}


def _cols_for(chunk):
    return 4 if chunk[4] == "wsplit" else 3   # St,So,wt,wo | St,So,w


def _dma_order():
    """t,o adjacent per chunk; last chunk's t pulled before the
    second-to-last chunk's o so its exp(t) clears before the end."""
    n = len(CHUNKS)
    order = []
    for i in range(n):
        order += [(i, "t"), (i, "o")]
    item, before = (n - 1, "t"), (n - 2, "o")
    order.remove(item)
    i = order.index(before)
    order.insert(i, item)
    for ci in reversed(HOIST):
        order.remove((ci, "t"))
        order.insert(2, (ci, "t"))
    return order


def _ttr(nc, out_ap, in0_ap, in1_ap, accum_ap):
    nc.vector._custom_dve(
        TENSOR_TENSOR_REDUCE, out=out_ap, in0=in0_ap, in1=in1_ap,
        s0=0.0, s1=1.0, accum_out=accum_ap,
    )


def _build_nc():
    n = len(CHUNKS)
    dma_order = _dma_order()
    for tr0 in set(c[0] for c in CHUNKS):
        assert sum(c[3] for c in CHUNKS if c[0] == tr0) == BINS

    cols = []
    col = 0
    for c in CHUNKS:
        cols.append(col)
        col += _cols_for(c)
    ncols = col

    nc = bacc.Bacc(
        "TRN2", target_bir_lowering=False, debug=False, num_devices=NCORES
    )
    o_ap = nc.dram_tensor("o_in", [ROWS, BINS], F32, kind="ExternalInput").ap()
    t_ap = nc.dram_tensor("t_in", [ROWS, BINS], F32, kind="ExternalInput").ap()
    s_ap = nc.dram_tensor("stats", [P, ncols], F32, kind="ExternalOutput").ap()

    # big: raw bump-allocated SBUF stats tensor (tracked by name).
    # store_src: a manual-range ALIAS of big's bytes that no instruction ever
    # writes. The store-prep reads the alias, so Tile gives it NO data deps
    # and its ~1us SWDGE descriptor-gen runs ~1us into the program instead of
    # after the last compute. The descriptors only encode ADDRESSES -- the
    # DMA reads big's bytes when the trigger fires, and the trigger is gated
    # on the final compute via signals_writable, so HW ordering is correct.
    big_h = nc.alloc_sbuf_tensor("big_stats", [P, ncols], F32)
    big = big_h.ap()
    store_src = nc.alloc_sbuf_tensor_at(
        "store_src", [P, ncols], F32, offset=nc.lookup_mloc(big_h).addr
    ).ap()
    ctx = nc.alloc_sbuf_tensor("kvw_ctx", [P, 1], I32).ap()

    with tile.TileContext(nc) as tc:
        with (
            tc.tile_pool(name="io", bufs=BUFS_IO) as io,
            tc.tile_pool(name="work", bufs=BUFS_WORK) as work,
        ):
            nc.gpsimd.memset(ctx[:, :], 0)
            # Store-prep emitted FIRST: reads the never-written alias, so its
            # only dep is the ctx memset (same Pool queue) -- desc-gen runs
            # at ~1us, entirely off the critical path.
            dmasw0 = tc.sems[PROC_NAME_TO_IDX["DMASW0"]]
            in4 = store_src[:, :].rearrange("p (a b c) -> p a b c", a=1, b=1)
            out4 = s_ap[:, :].rearrange("(a p) (b c) -> a p b c", a=1, b=1)
            nc.gpsimd.kv_writeback(
                out4, in4, ctx[:, :], prepare_only=True, sem=dmasw0
            )

            tiles = {}
            pre_done = set()
            wt_done = set()
            emitted = 0
            ets = {}
            last_eo = [None]
            last_prod = [None]

            def emit_wt(ci):
                r0, R, c0, CS, m = CHUNKS[ci][:5]
                col = cols[ci]
                t_t = tiles[(ci, "t")]
                et = ets[ci]
                p1 = work.tile([P, BINS], F32, tag="diff")
                _ttr(nc, p1[:R, :CS], et[:R, :CS], t_t[:R, :CS],
                     big[:R, col + 2:col + 3])
                last_prod[0] = p1
                wt_done.add(ci)

            def emit_t_part(ci):
                r0, R, c0, CS, m = CHUNKS[ci][:5]
                col = cols[ci]
                t_t = tiles[(ci, "t")]
                if ci in HOIST:
                    et = work.tile([P, CS], F32, tag=f"et{ci}", bufs=1,
                                   name=f"et{ci}")
                else:
                    et = work.tile([P, BINS], F32, tag="et")
                nc.scalar.activation(
                    et[:R, :CS], t_t[:R, :CS], Exp,
                    accum_out=big[:R, col:col + 1],
                )
                ets[ci] = et
                if m == "wsplit" and _flags(CHUNKS[ci]).get("pre_wt", True):
                    emit_wt(ci)

            def emit_rest(ci):
                r0, R, c0, CS, m = CHUNKS[ci][:5]
                col = cols[ci]
                t_t = tiles[(ci, "t")]
                o_t = tiles[(ci, "o")]
                if ci not in ets:
                    emit_t_part(ci)
                et = ets[ci]
                eo = work.tile([P, BINS], F32, tag="eo")
                nc.scalar.activation(
                    eo[:R, :CS], o_t[:R, :CS], Exp,
                    accum_out=big[:R, col + 1:col + 2],
                )
                last_eo[0] = eo
                if m == "pool":
                    diff = work.tile([P, BINS], F32, tag="diff")
                    nc.gpsimd.tensor_sub(diff[:R, :CS], t_t[:R, :CS],
                                         o_t[:R, :CS])
                    prod = work.tile([P, BINS], F32, tag="prod")
                    _ttr(nc, prod[:R, :CS], et[:R, :CS], diff[:R, :CS],
                         big[:R, col + 2:col + 3])
                    last_prod[0] = prod
                else:  # wsplit
                    if ci not in wt_done:
                        emit_wt(ci)
                    p2 = work.tile([P, BINS], F32, tag="prod")
                    _ttr(nc, p2[:R, :CS], et[:R, :CS], o_t[:R, :CS],
                         big[:R, col + 3:col + 4])
                    last_prod[0] = p2

            for (ci, which) in dma_order:
                r0, R, c0, CS, m = CHUNKS[ci][:5]
                ap = t_ap if which == "t" else o_ap
                key = (ci, which)
                if which == "t" and ci in HOIST:
                    tl = io.tile([P, CS], F32, tag=f"t{ci}", bufs=1,
                                 name=f"t{ci}")
                else:
                    tl = io.tile([P, BINS], F32, tag=f"{which}_t")
                tiles[key] = tl
                nc.sync.dma_start(tl[:R, :CS], ap[r0:r0 + R, c0:c0 + CS])
                if (
                    _flags(CHUNKS[ci]).get("pre_t")
                    and which == "t" and ci not in pre_done
                ):
                    emit_t_part(ci)
                    pre_done.add(ci)
                while (
                    emitted < n
                    and (emitted, "t") in tiles
                    and (emitted, "o") in tiles
                ):
                    emit_rest(emitted)
                    emitted += 1
            assert emitted == n
            # The trigger declares 1-element writes into the final eo/prod
            # work tiles: the WAW edges become an EventSemaphore wait (last
            # ACT exp tick + last DVE cTTR tick) right before the trigger on
            # the in-order Pool queue, so the store cannot fire before every
            # stats column is written. Do NOT name a tensor the prep reads
            # here -- that creates a self-deadlock cycle (trigger waits on
            # the prep's deferred-read DMASW tick, which only fires when the
            # trigger itself fires).
            nc.gpsimd.trigger_dma(
                count=None,
                signals_writable=[last_eo[0][:1, :1], last_prod[0][:1, :1]],
            )
    nc.compile()
    return nc


def kernel(output, target):
    output = np.ascontiguousarray(output, dtype=np.float32)
    target = np.ascontiguousarray(target, dtype=np.float32)
    assert output.shape == (B, J, D, BINS) and target.shape == (B, J, D, BINS)

    if "nc" not in _cache:
        _cache["nc"] = _build_nc()
    nc = _cache["nc"]

    in_maps = []
    for c in range(NCORES):
        sl = slice(c * BS, (c + 1) * BS)
        in_maps.append(
            {
                "o_in": output[sl].reshape(ROWS, BINS),
                "t_in": target[sl].reshape(ROWS, BINS),
            }
        )

    res = run_bass_kernel_spmd(nc, in_maps, list(range(NCORES)))
    _cache["last_results"] = res

    # host-side decode + final reduction (float64)
    per_row = np.empty((NCORES, ROWS), dtype=np.float64)
    for c in range(NCORES):
        st = res.results[c]["stats"].astype(np.float64)
        St = np.zeros(ROWS)
        So = np.zeros(ROWS)
        w = np.zeros(ROWS)
        col = 0
        for chunk in CHUNKS:
            r0, R, c0, CS, m = chunk[:5]
            rows = slice(r0, r0 + R)
            St[rows] += st[:R, col]
            So[rows] += st[:R, col + 1]
            if m == "wsplit":
                w[rows] += st[:R, col + 2] - st[:R, col + 3]
            else:
                w[rows] += st[:R, col + 2]
            col += _cols_for(chunk)
        per_row[c] = w / St + np.log(So) - np.log(St)

    per_row = per_row.reshape(B, J * D) / BINS          # per_bd, mean over bins
    per_jd = per_row.mean(axis=0)                        # [J*D]
    loss = per_jd.reshape(J, D).sum(axis=1)              # [J]
    return np.float32(loss.min())


# revision 26
# speedup vs baseline: 1.0008x; 1.0008x over previous
"""KLDiscretLoss joints kernel for TRN2 (8 NeuronCores, Bass/Tile).

Math: for each row (b,j,d) of BINS logits,
  kl_row_sum = sum_bins labels*(log_labels - log_scores)
             = w/St + log(So) - log(St)
  where St = sum(exp(t)), So = sum(exp(o)), w = sum(exp(t)*(t-o)).
(no max-subtraction needed: randn inputs, |x| <~ 6, exp is safe in f32)

Sharding: data-parallel over batch, 32 batches/core -> 1088 rows/core.
Device streams both tensors once (memory-bound, ~49.5us DMA roofline
per core) and emits per-row partial stats; host does the per-row
combine + batch-mean + sum-over-d + min-over-j in float64.

Schedule notes (TimelineSim cost model: 55412ns vs 56782ns baseline;
breakdown: 1966 prologue + 49514 gapless stream + 2231 compute drain
+ 107 trigger/store + 900 DMA-sem prop + 694 exit barrier):
- w's multiply+row-sum is ONE custom-DVE op (dve_ops.TENSOR_TENSOR_
  REDUCE uop: out = in0*in1, accum_out = row sum). The native fused
  TENSOR_TENSOR_REDUCE ISA opcode crashes this HW path
  (NRT_EXEC_UNIT_UNRECOVERABLE) but the CUSTOM_DVE_ANT uop-engine
  version runs fine and halves DVE cost vs separate mul+reduce.
- Bulk chunks ("pool"): GpSimd computes t-o, DVE does the fused
  mul+reduce. Tail chunks ("wsplit"): w = sum(et*t) - sum(et*o) as
  two fused ops -- the t-side runs as soon as t lands (+900ns DMA
  sem), so after the final o only sum(et*o) + exp(o) remain.
- The stats store is a SWDGE prepare+trigger kv_writeback (a plain
  SBUF->DRAM [128, ncols] copy; batch=1/dhi=128/dho=1/ncn=n_ctx).
  Descriptors encode ADDRESSES only, so desc-gen (~1us fixed SWDGE
  cost) is legal before the data exists; Tile would still gate the
  prep on big's writers, so the prep instead reads store_src, a
  manual-range ALIAS of big's bytes that nothing ever writes
  (bump-allocated and manual tensors are not range-fenced against
  each other). The prep then has no data deps and desc-gen runs ~1us
  into the program. After the last compute only ~107ns remain
  (gating EventSemaphore + Pool trigger decode + 9ns transfer) vs
  ~1330ns for a HWDGE dma_start store or ~1100ns for a data-gated
  prep.
- The trigger is ordered after the final ACT exp and final DVE cTTR
  via signals_writable on the last eo/prod work tiles (WAW edges ->
  EventSemaphore waits on both engines' final ticks; in-order
  engines make those cover ALL stats writers). Never pass a tensor
  the prep reads to signals_writable (self-deadlock: the trigger
  would wait on the prep's DMASW read-tick, which only fires when
  the trigger itself fires). sem= must be Tile's DMASW0 lane
  semaphore (tc.sems[...]) or the exit barrier deadlocks. accum_out
  OVERWRITES, so big needs no memset (runt rows 64:128 of its
  columns are garbage; never read by the host decode).
- Tail taper [800, 480, 768] + t12-before-o11 stream order are from
  a local search; ACT (exp+accum: 0.833ns/col + 372ns/op) and DVE
  (cTTR: ~1.04ns/col + ~105ns/op) are both ~saturated over the last
  ~6us of the stream, so the ~2.2us drain is the joint throughput
  floor of the two engines, not a scheduling gap.
- The runt tile (64 rows) is processed early where engines have
  slack: engine cost scales with free size (columns), not rows, so a
  late runt is maximum compute per streamed byte -- the worst tail.
- bufs io=6/work=3 keep the 49.5us DMA stream completely gapless.
"""

import numpy as np

import concourse.bass as bass
import concourse.tile as tile
from concourse import bacc, mybir
from concourse.bass_utils import run_bass_kernel_spmd
from concourse.dve_ops import TENSOR_TENSOR_REDUCE
from concourse.tile_sem_assignment import PROC_NAME_TO_IDX

B, J, D, BINS = 256, 17, 2, 2048
NCORES = 8
BS = B // NCORES
ROWS = BS * J * D              # 1088 rows per core
P = 128
F32 = mybir.dt.float32
I32 = mybir.dt.int32
Exp = mybir.ActivationFunctionType.Exp

# Chunk spec: (r0, R, c0, CS, mode[, flags]). Process order = list order.
#   "pool":   sub on GpSimd, fused mul+reduce on DVE   (cols: St,So,w)
#   "wsplit": w = ttr(et,t) - ttr(et,o), no sub        (cols: St,So,wt,wo)
# flags: pre_t  = emit exp(t) (+wt unless pre_wt=False) at t-DMA issue
#        pre_wt = False defers the wt cTTR to the chunk's main emission
_PRE = dict(pre_t=True)
CHUNKS = [
    # Bulk pool chunks as 1024-col pairs: finer Pool-sub/DVE-cTTR lumps
    # pipeline better mid-stream (-42ns vs full-2048 chunks; merging a
    # 1024-pair the other way costs +2638ns).
    (0,    128, 0,    1024, "pool"),
    (0,    128, 1024, 1024, "pool"),
    (1024,  64, 0, 2048, "pool"),      # runt early: engines have slack
    (128,  128, 0,    1024, "pool"),
    (128,  128, 1024, 1024, "pool"),
    (256,  128, 0,    1024, "pool"),
    (256,  128, 1024, 1024, "pool"),
    (384,  128, 0,    1024, "pool"),
    (384,  128, 1024, 1024, "pool"),
    (512,  128, 0,    1024, "pool"),
    (512,  128, 1024, 1024, "pool"),
    (640,  128, 0,    1024, "pool"),
    (640,  128, 1024, 1024, "pool"),
    (768,  128, 0,    1024, "wsplit", _PRE),
    (768,  128, 1024, 1024, "wsplit", _PRE),
    (896,  128, 0,    800,  "wsplit", _PRE),
    (896,  128, 800,  480,  "wsplit", _PRE),
    (896,  128, 1280, 768,  "wsplit", dict(pre_t=True, pre_wt=False)),
]
BUFS_IO = 6
BUFS_WORK = 3
# Hoisting tail-chunk t-loads to the stream front was measured WORSE
# (+230..+470ns): the stream is a fixed-rate pipe, so early hoisted loads
# displace chunks 8-10's arrivals INTO the saturated tail window, costing
# more than the removed wt cTTRs save. Keep empty.
HOIST = ()

_cache = {}


def _flags(chunk):
    return chunk[5] if len(chunk) > 5 else {}


def _cols_for(chunk):
    return 4 if chunk[4] == "wsplit" else 3   # St,So,wt,wo | St,So,w


def _dma_order():
    """t,o adjacent per chunk; last chunk's t pulled before the
    second-to-last chunk's o so its exp(t) clears before the end."""
    n = len(CHUNKS)
    order = []
    for i in range(n):
        order += [(i, "t"), (i, "o")]
    item, before = (n - 1, "t"), (n - 2, "o")
    order.remove(item)
    i = order.index(before)
    order.insert(i, item)
    for ci in reversed(HOIST):
        order.remove((ci, "t"))
        order.insert(2, (ci, "t"))
    return order


def _ttr(nc, out_ap, in0_ap, in1_ap, accum_ap):
    nc.vector._custom_dve(
        TENSOR_TENSOR_REDUCE, out=out_ap, in0=in0_ap, in1=in1_ap,
        s0=0.0, s1=1.0, accum_out=accum_ap,
    )


def _build_nc():
    n = len(CHUNKS)
    dma_order = _dma_order()
    for tr0 in set(c[0] for c in CHUNKS):
        assert sum(c[3] for c in CHUNKS if c[0] == tr0) == BINS

    cols = []
    col = 0
    for c in CHUNKS:
        cols.append(col)
        col += _cols_for(c)
    ncols = col

    nc = bacc.Bacc(
        "TRN2", target_bir_lowering=False, debug=False, num_devices=NCORES
    )
    o_ap = nc.dram_tensor("o_in", [ROWS, BINS], F32, kind="ExternalInput").ap()
    t_ap = nc.dram_tensor("t_in", [ROWS, BINS], F32, kind="ExternalInput").ap()
    s_ap = nc.dram_tensor("stats", [P, ncols], F32, kind="ExternalOutput").ap()

    # big: raw bump-allocated SBUF stats tensor (tracked by name).
    # store_src: a manual-range ALIAS of big's bytes that no instruction ever
    # writes. The store-prep reads the alias, so Tile gives it NO data deps
    # and its ~1us SWDGE descriptor-gen runs ~1us into the program instead of
    # after the last compute. The descriptors only encode ADDRESSES -- the
    # DMA reads big's bytes when the trigger fires, and the trigger is gated
    # on the final compute via signals_writable, so HW ordering is correct.
    big_h = nc.alloc_sbuf_tensor("big_stats", [P, ncols], F32)
    big = big_h.ap()
    store_src = nc.alloc_sbuf_tensor_at(
        "store_src", [P, ncols], F32, offset=nc.lookup_mloc(big_h).addr
    ).ap()
    ctx = nc.alloc_sbuf_tensor("kvw_ctx", [P, 1], I32).ap()

    with tile.TileContext(nc) as tc:
        with (
            tc.tile_pool(name="io", bufs=BUFS_IO) as io,
            tc.tile_pool(name="work", bufs=BUFS_WORK) as work,
        ):
            nc.gpsimd.memset(ctx[:, :], 0)
            # Store-prep emitted FIRST: reads the never-written alias, so its
            # only dep is the ctx memset (same Pool queue) -- desc-gen runs
            # at ~1us, entirely off the critical path.
            dmasw0 = tc.sems[PROC_NAME_TO_IDX["DMASW0"]]
            in4 = store_src[:, :].rearrange("p (a b c) -> p a b c", a=1, b=1)
            out4 = s_ap[:, :].rearrange("(a p) (b c) -> a p b c", a=1, b=1)
            nc.gpsimd.kv_writeback(
                out4, in4, ctx[:, :], prepare_only=True, sem=dmasw0
            )

            tiles = {}
            pre_done = set()
            wt_done = set()
            emitted = 0
            ets = {}
            last_eo = [None]
            last_prod = [None]

            def emit_wt(ci):
                r0, R, c0, CS, m = CHUNKS[ci][:5]
                col = cols[ci]
                t_t = tiles[(ci, "t")]
                et = ets[ci]
                p1 = work.tile([P, BINS], F32, tag="diff")
                _ttr(nc, p1[:R, :CS], et[:R, :CS], t_t[:R, :CS],
                     big[:R, col + 2:col + 3])
                last_prod[0] = p1
                wt_done.add(ci)

            def emit_t_part(ci):
                r0, R, c0, CS, m = CHUNKS[ci][:5]
                col = cols[ci]
                t_t = tiles[(ci, "t")]
                if ci in HOIST:
                    et = work.tile([P, CS], F32, tag=f"et{ci}", bufs=1,
                                   name=f"et{ci}")
                else:
                    et = work.tile([P, BINS], F32, tag="et")
                nc.scalar.activation(
                    et[:R, :CS], t_t[:R, :CS], Exp,
                    accum_out=big[:R, col:col + 1],
                )
                ets[ci] = et
                if m == "wsplit" and _flags(CHUNKS[ci]).get("pre_wt", True):
                    emit_wt(ci)

            def emit_rest(ci):
                r0, R, c0, CS, m = CHUNKS[ci][:5]
                col = cols[ci]
                t_t = tiles[(ci, "t")]
                o_t = tiles[(ci, "o")]
                if ci not in ets:
                    emit_t_part(ci)
                et = ets[ci]
                eo = work.tile([P, BINS], F32, tag="eo")
                nc.scalar.activation(
                    eo[:R, :CS], o_t[:R, :CS], Exp,
                    accum_out=big[:R, col + 1:col + 2],
                )
                last_eo[0] = eo
                if m == "pool":
                    diff = work.tile([P, BINS], F32, tag="diff")
                    nc.gpsimd.tensor_sub(diff[:R, :CS], t_t[:R, :CS],
                                         o_t[:R, :CS])
                    prod = work.tile([P, BINS], F32, tag="prod")
                    _ttr(nc, prod[:R, :CS], et[:R, :CS], diff[:R, :CS],
                         big[:R, col + 2:col + 3])
                    last_prod[0] = prod
                else:  # wsplit
                    if ci not in wt_done:
                        emit_wt(ci)
                    p2 = work.tile([P, BINS], F32, tag="prod")
                    _ttr(nc, p2[:R, :CS], et[:R, :CS], o_t[:R, :CS],
                         big[:R, col + 3:col + 4])
                    last_prod[0] = p2

            for (ci, which) in dma_order:
                r0, R, c0, CS, m = CHUNKS[ci][:5]
                ap = t_ap if which == "t" else o_ap
                key = (ci, which)
                if which == "t" and ci in HOIST:
                    tl = io.tile([P, CS], F32, tag=f"t{ci}", bufs=1,
                                 name=f"t{ci}")
                else:
                    tl = io.tile([P, BINS], F32, tag=f"{which}_t")
                tiles[key] = tl
                nc.sync.dma_start(tl[:R, :CS], ap[r0:r0 + R, c0:c0 + CS])
                if (
                    _flags(CHUNKS[ci]).get("pre_t")
                    and which == "t" and ci not in pre_done
                ):
                    emit_t_part(ci)
                    pre_done.add(ci)
                while (
                    emitted < n
                    and (emitted, "t") in tiles
                    and (emitted, "o") in tiles
                ):
                    emit_rest(emitted)
                    emitted += 1
            assert emitted == n
            # The trigger declares 1-element writes into the final eo/prod
            # work tiles: the WAW edges become an EventSemaphore wait (last
            # ACT exp tick + last DVE cTTR tick) right before the trigger on
            # the in-order Pool queue, so the store cannot fire before every
            # stats column is written. Do NOT name a tensor the prep reads
            # here -- that creates a self-deadlock cycle (trigger waits on
            # the prep's deferred-read DMASW tick, which only fires when the
            # trigger itself fires).
            nc.gpsimd.trigger_dma(
                count=None,
                signals_writable=[last_eo[0][:1, :1], last_prod[0][:1, :1]],
            )
    nc.compile()
    return nc


def kernel(output, target):
    output = np.ascontiguousarray(output, dtype=np.float32)
    target = np.ascontiguousarray(target, dtype=np.float32)
    assert output.shape == (B, J, D, BINS) and target.shape == (B, J, D, BINS)

    if "nc" not in _cache:
        _cache["nc"] = _build_nc()
    nc = _cache["nc"]

    in_maps = []
    for c in range(NCORES):
        sl = slice(c * BS, (c + 1) * BS)
        in_maps.append(
            {
                "o_in": output[sl].reshape(ROWS, BINS),
                "t_in": target[sl].reshape(ROWS, BINS),
            }
        )

    res = run_bass_kernel_spmd(nc, in_maps, list(range(NCORES)))
    _cache["last_results"] = res

    # host-side decode + final reduction (float64)
    per_row = np.empty((NCORES, ROWS), dtype=np.float64)
    for c in range(NCORES):
        st = res.results[c]["stats"].astype(np.float64)
        St = np.zeros(ROWS)
        So = np.zeros(ROWS)
        w = np.zeros(ROWS)
        col = 0
        for chunk in CHUNKS:
            r0, R, c0, CS, m = chunk[:5]
            rows = slice(r0, r0 + R)
            St[rows] += st[:R, col]
            So[rows] += st[:R, col + 1]
            if m == "wsplit":
                w[rows] += st[:R, col + 2] - st[:R, col + 3]
            else:
                w[rows] += st[:R, col + 2]
            col += _cols_for(chunk)
        per_row[c] = w / St + np.log(So) - np.log(St)

    per_row = per_row.reshape(B, J * D) / BINS          # per_bd, mean over bins
    per_jd = per_row.mean(axis=0)                        # [J*D]
    loss = per_jd.reshape(J, D).sum(axis=1)              # [J]
    return np.float32(loss.min())
